# revision 1
# baseline (speedup 1.0000x reference)
"""Trainium2 Bass kernel for Conformer-style relative-position MHSA.

Sharding: data-parallel over batch — B=8 batch elements, one per NeuronCore.
Each core computes LN -> QKVP projections -> rel-pos attention scores
(Transformer-XL shift done via a strided DRAM round-trip: write rows with
stride 1025, read windows with stride 1024) -> softmax -> AV -> output
projection -> residual, entirely for its batch element. No collectives.

Host-side folds: 1/sqrt(64) into key/pos kernels, LN gamma into q/k/v
kernels, projection bias added on host after gather.
"""

import sys

for _p in ("/opt/trn_rl_repo", "/root/.axon_site/_ro/pypackages"):
    if _p not in sys.path:
        sys.path.insert(0, _p)

import numpy as np
import ml_dtypes

import concourse.bass as bass
import concourse.mybir as mybir
import concourse.tile as tile
from concourse import bacc
from concourse.bass_utils import run_bass_kernel_spmd
from concourse.masks import make_identity

F32 = mybir.dt.float32
BF16 = mybir.dt.bfloat16
FP16 = mybir.dt.float16
AX = mybir.AluOpType
AF = mybir.ActivationFunctionType

P = 128          # partitions
T = 1024         # sequence length
D = 512          # model dim
H = 8            # heads
O = 64           # head size
KT = D // P      # 4 k-tiles over model dim
NT = T // P      # 8 tiles over sequence
NCH = T // 512   # 2 free-dim chunks of 512
LN_EPS = 1e-3


def build_nc():
    nc = bacc.Bacc("TRN2", target_bir_lowering=False)

    # ---- per-core DRAM I/O ----
    x_res = nc.dram_tensor("x_res", [P, NT, D], F32, kind="ExternalInput")
    post = nc.dram_tensor("post", [P, KT, T], BF16, kind="ExternalInput")
    wq = nc.dram_tensor("wq", [P, KT, D], BF16, kind="ExternalInput")
    wk = nc.dram_tensor("wk", [P, KT, D], BF16, kind="ExternalInput")
    wv = nc.dram_tensor("wv", [P, KT, D], BF16, kind="ExternalInput")
    wp = nc.dram_tensor("wp", [P, KT, D], BF16, kind="ExternalInput")
    wo = nc.dram_tensor("wo", [P, KT, D], BF16, kind="ExternalInput")
    u_in = nc.dram_tensor("u_in", [P, KT], F32, kind="ExternalInput")
    v_in = nc.dram_tensor("v_in", [P, KT], F32, kind="ExternalInput")
    beta_in = nc.dram_tensor("beta_in", [P, D], BF16, kind="ExternalInput")
    out = nc.dram_tensor("out", [T, D], F32, kind="ExternalOutput")

    with tile.TileContext(nc) as tc:
        with (
            tc.tile_pool(name="consts", bufs=1) as consts,
            tc.tile_pool(name="acts", bufs=1) as acts,
            tc.tile_pool(name="dram", bufs=2, space="DRAM") as dram_pool,
        ):
            # residual input + LayerNorm first (weights stream in behind)
            xres_sb = acts.tile([P, NT, D], F32)
            nc.sync.dma_start(xres_sb[:], x_res[:])
            beta_sb = consts.tile([P, D], BF16, tag="beta")
            nc.sync.dma_start(beta_sb[:], beta_in[:])
            eps_sb = consts.tile([P, 1], F32, tag="eps")
            nc.vector.memset(eps_sb[:], LN_EPS)
            ident = consts.tile([P, P], BF16)
            make_identity(nc, ident)
            ones_bc = consts.tile([P, O], BF16, tag="ones_bc")
            nc.vector.memset(ones_bc[:], 1.0)

            # long-lived activation tiles
            qu = acts.tile([P, KT, T], BF16)
            qv = acts.tile([P, KT, T], BF16)
            kT_sb = acts.tile([P, KT, T], BF16)
            pT_sb = acts.tile([P, KT, T], BF16)
            outT = acts.tile([P, KT, T], BF16)
            avw = []
            for h in range(H):
                t_ = acts.tile([P, NT, O + 2], BF16, tag=f"avw{h}")
                avw.append(t_)
                nc.vector.memset(t_[:], 1.0)

            with (
                tc.tile_pool(name="early", bufs=1) as early,
                tc.tile_pool(name="psP", bufs=6, space="PSUM") as psP,
                tc.tile_pool(name="psB", bufs=2, space="PSUM") as psB,
            ):
                xlnT = early.tile([P, KT, T], BF16)
                xln_nd = early.tile([P, NT, D], BF16)
                with tc.tile_pool(name="ln_tmp", bufs=4) as ln_tmp:
                    with nc.named_scope("ln"):
                        for nt in range(NT):
                            st6 = ln_tmp.tile([P, 6], F32, tag="st6")
                            nc.vector.bn_stats(out=st6[:], in_=xres_sb[:, nt, :])
                            mv = ln_tmp.tile([P, 2], F32, tag="mv")
                            nc.vector.bn_aggr(out=mv[:], in_=st6[:])
                            sd = ln_tmp.tile([P, 1], F32, tag="sd")
                            nc.scalar.activation(out=sd[:], in_=mv[:, 1:2],
                                                 func=AF.Sqrt, bias=eps_sb[:])
                            rstd = ln_tmp.tile([P, 1], F32, tag="rstd")
                            nc.vector.reciprocal(rstd[:], sd[:])
                            nc.vector.tensor_scalar(
                                out=xln_nd[:, nt, :], in0=xres_sb[:, nt, :],
                                scalar1=mv[:, 0:1], scalar2=rstd[:],
                                op0=AX.subtract, op1=AX.mult)
                            nc.vector.tensor_add(
                                xln_nd[:, nt, :], xln_nd[:, nt, :], beta_sb[:])
                        for kt in range(KT):
                            ps_x = psB.tile([P, T], BF16, tag="tx")
                            for nt in range(NT):
                                nc.tensor.transpose(
                                    ps_x[:, bass.ts(nt, P)],
                                    xln_nd[:, nt, bass.ts(kt, P)],
                                    ident[:])
                            nc.scalar.copy(xlnT[:, kt, :], ps_x[:])

                # weights (emitted after LN so xres/LN win the DMA queue)
                post_sb = early.tile([P, KT, T], BF16)
                nc.sync.dma_start(post_sb[:], post[:])
                w_sb = {}
                for name, t in (("wq", wq), ("wk", wk), ("wv", wv), ("wp", wp),
                                ("wo", wo)):
                    w_sb[name] = consts.tile([P, KT, D], BF16, tag=f"w_{name}",
                                             name=f"w_{name}")
                    nc.sync.dma_start(w_sb[name][:], t[:])
                u_sb = consts.tile([P, KT], F32, tag="u")
                nc.sync.dma_start(u_sb[:], u_in[:])
                v_sb = consts.tile([P, KT], F32, tag="v")
                nc.sync.dma_start(v_sb[:], v_in[:])

                # ---- projections (kt-outer for stationary reuse) ----
                with nc.named_scope("proj"):
                    for mch in range(KT):
                        ps_q = [psP.tile([P, 512], F32, tag="ps", name="ps")
                                for _ in range(NCH)]
                        for kt in range(KT):
                            for nch in range(NCH):
                                nc.tensor.matmul(
                                    ps_q[nch][:],
                                    w_sb["wq"][:, kt, bass.ts(mch, P)],
                                    xlnT[:, kt, bass.ts(nch, 512)],
                                    start=(kt == 0), stop=(kt == KT - 1))
                        for nch in range(NCH):
                            nc.scalar.add(qu[:, mch, bass.ts(nch, 512)],
                                          ps_q[nch][:], u_sb[:, mch:mch + 1])
                            nc.scalar.add(qv[:, mch, bass.ts(nch, 512)],
                                          ps_q[nch][:], v_sb[:, mch:mch + 1])
                        ps_k = [psP.tile([P, 512], F32, tag="ps", name="ps")
                                for _ in range(NCH)]
                        for kt in range(KT):
                            for nch in range(NCH):
                                nc.tensor.matmul(
                                    ps_k[nch][:],
                                    w_sb["wk"][:, kt, bass.ts(mch, P)],
                                    xlnT[:, kt, bass.ts(nch, 512)],
                                    start=(kt == 0), stop=(kt == KT - 1))
                        for nch in range(NCH):
                            nc.vector.tensor_copy(
                                kT_sb[:, mch, bass.ts(nch, 512)], ps_k[nch][:])
                        ps_p = [psP.tile([P, 512], F32, tag="ps", name="ps")
                                for _ in range(NCH)]
                        for kt in range(KT):
                            for nch in range(NCH):
                                nc.tensor.matmul(
                                    ps_p[nch][:],
                                    w_sb["wp"][:, kt, bass.ts(mch, P)],
                                    post_sb[:, kt, bass.ts(nch, 512)],
                                    start=(kt == 0), stop=(kt == KT - 1))
                        for nch in range(NCH):
                            nc.vector.tensor_copy(
                                pT_sb[:, mch, bass.ts(nch, 512)], ps_p[nch][:])
                    for mt in range(NT):
                        ps_v = psP.tile([P, 512], F32, tag="ps", name="ps")
                        for kt in range(KT):
                            nc.tensor.matmul(
                                ps_v[:],
                                xlnT[:, kt, bass.ts(mt, P)],
                                w_sb["wv"][:, kt, :],
                                start=(kt == 0), stop=(kt == KT - 1))
                        for h in range(H):
                            nc.scalar.copy(avw[h][:, mt, 0:O],
                                           ps_v[:, bass.ts(h, O)])

            # ====== attention: software-pipelined across head pairs ==========
            with (
                tc.tile_pool(name="ywr", bufs=4) as ywr_pool,
                tc.tile_pool(name="bds", bufs=5) as bds_pool,
                tc.tile_pool(name="sfull", bufs=1) as s_pool,
                tc.tile_pool(name="et", bufs=1) as et_pool,
                tc.tile_pool(name="avsb", bufs=3) as avsb_pool,
                tc.tile_pool(name="ps_bd", bufs=2, space="PSUM") as ps_bd_pool,
                tc.tile_pool(name="ps_s", bufs=2, space="PSUM") as ps_s_pool,
                tc.tile_pool(name="ps_av", bufs=2, space="PSUM") as ps_av_pool,
                tc.tile_pool(name="psT", bufs=2, space="PSUM") as psT,
            ):
                NPAIR = H // 2
                ydram_all = {}
                s_all = {}
                et_all = {}

                def emit_bd_nt(pair, nt):
                    heads = (2 * pair, 2 * pair + 1)
                    ywr = {}
                    for h in heads:
                        ywr[h] = ywr_pool.tile(
                            [P, T + 1], FP16,
                            tag=f"ywr{h % 2}", name=f"ywr{h % 2}")
                        nc.gpsimd.memset(ywr[h][:, 0:1], 0.0)
                    for h in heads:
                        base = (h % 2) * O
                        ps_bd = [ps_bd_pool.tile([P, 512], F32, tag="ps",
                                                 name="ps")
                                 for _ in range(NCH)]
                        for mch in range(NCH):
                            nc.tensor.matmul(
                                ps_bd[mch][:],
                                qv[base:base + O, pair, bass.ts(nt, P)],
                                pT_sb[base:base + O, pair, bass.ts(mch, 512)],
                                start=True, stop=True)
                        nc.vector.tensor_copy(ywr[h][:, 1:513], ps_bd[0][:])
                        nc.scalar.copy(ywr[h][:, 513:1025], ps_bd[1][:])
                    for h in heads:
                        nc.gpsimd.dma_start(
                            ydram_all[pair][h][bass.ts(nt, P), :], ywr[h][:])

                def emit_acs_nt(pair, nt):
                    heads = (2 * pair, 2 * pair + 1)
                    bds = {}
                    for h in heads:
                        bds[h] = bds_pool.tile(
                            [P, T], FP16, tag=f"bds{h % 2}", name=f"bds{h % 2}")
                        yflat = ydram_all[pair][h].flatten()
                        start = T * (nt * P + 1)
                        nc.sync.dma_start(
                            bds[h][:],
                            yflat[start:start + P * T].rearrange(
                                "(a b) -> a b", b=T))
                    for h in heads:
                        base = (h % 2) * O
                        ps_s = [ps_s_pool.tile([P, 512], F32, tag="ps",
                                               name="ps")
                                for _ in range(NCH)]
                        for mch in range(NCH):
                            nc.tensor.matmul(
                                ps_s[mch][:],
                                qu[base:base + O, pair, bass.ts(nt, P)],
                                kT_sb[base:base + O, pair, bass.ts(mch, 512)],
                                start=True, stop=True)
                        for mch in range(NCH):
                            nc.vector.tensor_tensor(
                                out=s_all[pair][h][:, nt, bass.ts(mch, 512)],
                                in0=ps_s[mch][:],
                                in1=bds[h][:, bass.ts(mch, 512)],
                                op=AX.add)

                def emit_tx(pair, h, mt):
                    ps_t = psT.tile([P, T], BF16, tag="tx", name="ps_t")
                    for nt in range(NT):
                        nc.tensor.transpose(
                            ps_t[:, bass.ts(nt, P)],
                            s_all[pair][h][:, nt, bass.ts(mt, P)],
                            ident[:])
                    nc.scalar.activation(
                        out=et_all[pair][h][:, mt, :], in_=ps_t[:],
                        func=AF.Exp)

                av_ps = {}

                def emit_av_mt(pair, h, mt):
                    if (pair, h) not in av_ps:
                        av_ps[(pair, h)] = [
                            ps_av_pool.tile([P, 512], F32, tag="ps", name="ps")
                            for _ in range(NCH)]
                    ps_av = av_ps[(pair, h)]
                    et = et_all[pair]
                    for nch in range(NCH):
                        nc.tensor.matmul(
                            ps_av[nch][0:O + 1, :],
                            avw[h][:, mt, 0:O + 1],
                            et[h][:, mt, bass.ts(nch, 512)],
                            start=(mt == 0), stop=(mt == NT - 1))

                def emit_av_fin(pair, h):
                    base = (h % 2) * O
                    ps_av = av_ps.pop((pair, h))
                    for nch in range(NCH):
                        av_sb = avsb_pool.tile([O + 1, 512], BF16,
                                               tag=f"avsb{h % 2}")
                        nc.scalar.copy(av_sb[:], ps_av[nch][0:O + 1, :])
                        ps_bc = ps_bd_pool.tile([P, 512], F32, tag="ps",
                                                name="ps")
                        nc.tensor.matmul(
                            ps_bc[0:O, :],
                            ones_bc[O:O + 1, :],
                            av_sb[O:O + 1, :],
                            start=True, stop=True)
                        rb = avsb_pool.tile([O, 512], F32, tag=f"rb{h % 2}")
                        nc.vector.reciprocal_approx_fast(
                            out=rb[:], in_=ps_bc[0:O, :])
                        nc.gpsimd.tensor_tensor(
                            out=outT[base:base + O, pair, bass.ts(nch, 512)],
                            in0=av_sb[0:O, :],
                            in1=rb[:],
                            op=AX.mult)

                for p in range(NPAIR + 1):
                    if p < NPAIR:
                        heads = (2 * p, 2 * p + 1)
                        ydram_all[p] = {
                            h: dram_pool.tile([T, T + 1], FP16,
                                              tag=f"y{h % 2}", name=f"y{h % 2}")
                            for h in heads}
                        s_all[p] = {
                            h: s_pool.tile([P, NT, T], BF16,
                                           tag=f"s{h % 2}", name=f"s{h % 2}")
                            for h in heads}
                        et_all[p] = {
                            h: et_pool.tile([P, NT, T], BF16,
                                            tag=f"et{h % 2}", name=f"et{h % 2}")
                            for h in heads}
                    for i in range(NT):
                        if 1 <= p <= NPAIR:
                            emit_acs_nt(p - 1, i)
                        if p < NPAIR:
                            emit_bd_nt(p, i)
                    if 1 <= p <= NPAIR:
                        with nc.named_scope("tx"):
                            for h in (2 * (p - 1), 2 * (p - 1) + 1):
                                for mt in range(NT):
                                    emit_tx(p - 1, h, mt)
                        with nc.named_scope("avf"):
                            for h in (2 * (p - 1), 2 * (p - 1) + 1):
                                for mt in range(NT):
                                    emit_av_mt(p - 1, h, mt)
                                emit_av_fin(p - 1, h)

            # ---- output projection + residual ----
            with (
                tc.tile_pool(name="fin", bufs=4) as fin_pool,
                tc.tile_pool(name="ps_y", bufs=4, space="PSUM") as ps_y_pool,
            ):
                with nc.named_scope("out"):
                    for nt in range(NT):
                        ps_y = ps_y_pool.tile([P, D], F32, tag="ps", name="ps")
                        for c in range(KT):
                            nc.tensor.matmul(
                                ps_y[:],
                                outT[:, c, bass.ts(nt, P)],
                                w_sb["wo"][:, c, :],
                                start=(c == 0), stop=(c == KT - 1))
                        fin = fin_pool.tile([P, D], F32)
                        nc.vector.tensor_tensor(
                            out=fin[:], in0=ps_y[:], in1=xres_sb[:, nt, :],
                            op=AX.add)
                        nc.sync.dma_start(out[bass.ts(nt, P), :], fin[:])

    nc.compile()
    return nc


_NC = None


def _get_nc():
    global _NC
    if _NC is None:
        _NC = build_nc()
    return _NC


def _run(inputs_dict, trace=False, trace_cores=None):
    bf = ml_dtypes.bfloat16
    inputs = np.asarray(inputs_dict["inputs"], np.float32)
    pos = np.asarray(inputs_dict["pos"], np.float32)
    gamma = np.asarray(inputs_dict["gamma"], np.float32)
    beta = np.asarray(inputs_dict["beta"], np.float32)
    qk = np.asarray(inputs_dict["query_kernel"], np.float32)   # [H, D, O]
    kk = np.asarray(inputs_dict["key_kernel"], np.float32)
    vk = np.asarray(inputs_dict["value_kernel"], np.float32)
    pk = np.asarray(inputs_dict["pos_kernel"], np.float32)
    u = np.asarray(inputs_dict["pos_bias_u"], np.float32)      # [H, O]
    v = np.asarray(inputs_dict["pos_bias_v"], np.float32)
    prk = np.asarray(inputs_dict["projection_kernel"], np.float32)  # [H, O, D]
    pbias = np.asarray(inputs_dict["projection_bias"], np.float32)

    scale = 1.0 / np.sqrt(np.float32(O))

    def wcat(w, rowscale=None):  # [H, D, O] -> [P, KT, (h o)]
        c = np.transpose(w, (1, 0, 2)).reshape(D, H * O)   # [i, (h o)]
        if rowscale is not None:
            c = c * rowscale[:, None]
        return np.ascontiguousarray(
            c.reshape(KT, P, H * O).transpose(1, 0, 2)).astype(bf)

    wq_c = wcat(qk, gamma)
    wk_c = wcat(kk * scale, gamma)
    wv_c = wcat(vk, gamma)
    wp_c = wcat(pk * scale)
    wo_c = np.ascontiguousarray(
        prk.reshape(H * O, D).reshape(KT, P, D).transpose(1, 0, 2)).astype(bf)
    u_c = np.ascontiguousarray(u.reshape(H * O).reshape(KT, P).T).astype(np.float32)
    v_c = np.ascontiguousarray(v.reshape(H * O).reshape(KT, P).T).astype(np.float32)
    beta_adj = np.where(gamma != 0, beta / np.where(gamma == 0, 1, gamma), 0.0)
    beta_b = np.broadcast_to(beta_adj[None, :], (P, D)).astype(bf).copy()

    in_maps = []
    for b in range(8):
        x_b = inputs[b]
        in_maps.append({
            "x_res": np.ascontiguousarray(
                x_b.reshape(NT, P, D).transpose(1, 0, 2)).astype(np.float32),
            "post": np.ascontiguousarray(
                pos[b].T.reshape(KT, P, T).transpose(1, 0, 2)).astype(bf),
            "wq": wq_c, "wk": wk_c, "wv": wv_c, "wp": wp_c, "wo": wo_c,
            "u_in": u_c, "v_in": v_c, "beta_in": beta_b,
        })

    nc = _get_nc()
    res = run_bass_kernel_spmd(
        nc, in_maps, core_ids=list(range(8)), trace=trace,
        trace_cores=trace_cores)
    outs = np.stack([np.asarray(r["out"], np.float32) for r in res.results])
    outs = outs + pbias[None, None, :]
    return outs, res


def kernel(**inputs):
    outs, _ = _run(inputs)
    return outs


if __name__ == "__main__":
    nc = build_nc()
    print("built ok")

